# revision 15
# baseline (speedup 1.0000x reference)
"""Distributed GAT (AnomalyDAE encoder) kernel for 8 TRN2 NeuronCores.

Reference computation:
    h = leaky_relu(x @ W_dense.T + b_dense, 0.01)          # [N, 128]
    g = h @ W_gat.T                                        # [N, 64]
    a_src = g @ att_src ; a_dst = g @ att_dst              # [N]
    with self-loops appended, per edge (s -> d):
        e = leaky_relu(a_src[s] + a_dst[d], 0.2)
        alpha = segment_softmax(e, by d)
    out[d] = sum_e alpha_e * g[s_e] + b_gat                # [N, 64]

Sharding: nodes split contiguously across 8 cores (6250 each); edges
partitioned by destination core. Each core's local nodes are sorted by
in-degree on the host so 128-node tiles have near-uniform degree; per-tile
edge lists are padded to the tile max degree with host-built 0/1 masks
(replacing sentinel rows).

Device pipeline per core:
  node phase: per 128-node tile, matmuls against host-pretransposed x
    -> hT -> lrelu -> gT -> a_src/a_dst matvec -> PE-transpose -> table rows
    [g bf16(64) | a_src f32 | a_dst f32 | pad] (256 B) into one of two DRAM
    shard halves split by local position (pos < 3200 -> A).
  Two AllGathers (one per half): AG1 fires after node tiles 0-24 and
    overlaps the rest of the node phase; AG2 after tile 49.
  edge phase: two sweeps (A then B). Each sweep streams gpsimd dma_gather
    calls back-to-back over rotating SWDGE queues (Pool engine saturated),
    while DVE/ACT trail behind computing exp(LRelu) weights, the pad mask,
    and partial weighted sums into per-tile accumulators [P,64]+[P,1].
  Sweep B finalizes: out = (numA+numB) / (denA+denB) + b_gat.

Softmax is computed without the segment-max shift (logits are O(1); result
identical in exact arithmetic, and the two half partial sums compose
exactly).
"""

import numpy as np
import ml_dtypes

bf16 = ml_dtypes.bfloat16

R = 8            # cores
P = 128          # partitions / tile size
W_ROW = 128      # table row width in bf16 elems (256 B rows for dma_gather)
A_SRC_F32 = 32   # f32 column of a_src within a row (byte offset 128)
DCALL = 8        # gather rows per dma_gather call chunk (1024-desc ring)
NQ = 4           # SWDGE queues


class Cfg:
    def __init__(self, N, E, IN=512, EMB=128, OUT=64):
        assert N % R == 0
        self.N, self.E, self.IN, self.EMB, self.OUT = N, E, IN, EMB, OUT
        self.NL = N // R
        nlp = ((self.NL + 2 * P - 1) // (2 * P)) * (2 * P)
        self.NL_pad = nlp
        self.TILES = nlp // P
        self.H = nlp // 2                    # local-position half split
        self.HTILES = self.H // P
        self.NHALF = self.H * R              # rows per table half
        assert self.NHALF < 32768, "dma_gather int16 index limit"


CFG_REAL = Cfg(N=50000, E=1600000)


# --------------------------------------------------------------------------
# host-side preprocessing
# --------------------------------------------------------------------------

def _wrap_idx(lin):
    """dma_gather index layout: linear i -> [i % 16, i // 16], replicated
    across the 8 Q7 core groups -> [128, len/16] int16."""
    assert len(lin) % 16 == 0
    w = lin.reshape(-1, 16).T.astype(np.int16)
    return np.tile(w, (8, 1))


def _prepare(cfg, x, edge_index, W_dense, b_dense, W_gat, att_src, att_dst,
             b_gat):
    N, NL, NL_pad, TILES = cfg.N, cfg.NL, cfg.NL_pad, cfg.TILES
    src = edge_index[0].astype(np.int64)
    dst = edge_index[1].astype(np.int64)
    loops = np.arange(N, dtype=np.int64)
    src = np.concatenate([src, loops])
    dst = np.concatenate([dst, loops])

    deg = np.bincount(dst, minlength=N)

    pos_of = np.empty(N, dtype=np.int64)
    orders = []
    for r in range(R):
        dloc = deg[r * NL:(r + 1) * NL]
        order = np.argsort(-dloc, kind="stable")
        orders.append(order)
        pos_of[r * NL + order] = np.arange(NL)

    core_of_node = np.arange(N) // NL
    pos_src = pos_of[src]
    in_b = pos_src >= cfg.H                  # source in half B (by position)
    gid_half = core_of_node[src] * cfg.H + np.where(in_b, pos_src - cfg.H,
                                                    pos_src)

    # per-(core,position,half) degrees
    degh = np.zeros((2, R * NL_pad), dtype=np.int64)
    dst_key = core_of_node[dst] * NL_pad + pos_of[dst]
    np.add.at(degh[0], dst_key[~in_b], 1)
    np.add.at(degh[1], dst_key[in_b], 1)
    D_lists = []
    for h in range(2):
        dt_ = degh[h].reshape(R, TILES, P).max(axis=(0, 2))
        D_lists.append(np.maximum(dt_, 1).astype(np.int64))

    # sort edges by (dst position, half, gid)
    ekey = dst_key * 2 + in_b
    eorder = np.lexsort((gid_half, ekey))
    dst_key_s = dst_key[eorder]
    hb_s = in_b[eorder]
    gid_s = gid_half[eorder]
    # slot index within the (dst, half) group
    grp = dst_key_s * 2 + hb_s
    starts = np.zeros(2 * R * NL_pad + 1, dtype=np.int64)
    np.add.at(starts, grp + 1, 1)
    starts = np.cumsum(starts)
    k_of = np.arange(len(gid_s)) - starts[grp]

    # offs[h]: int32 [R, NL_pad, Dmax_h]; pad slots are masked out by msks[h].
    # Pad targets are SCATTERED across the table: a single shared pad row
    # makes whole gather calls hammer one 256-B row -> HBM channel hotspot
    # that serializes all 16 SDMA engines (~10x slowdown on those calls).
    offs, msks = [], []
    for h in range(2):
        Dmax = int(D_lists[h].max())
        o = ((np.arange(R)[:, None, None] * 3181
              + np.arange(NL_pad)[None, :, None] * 131
              + np.arange(Dmax)[None, None, :] * 7919) % cfg.NHALF
             ).astype(np.int32)
        m = np.zeros((R, NL_pad, Dmax), dtype=np.float32)
        sel = hb_s == bool(h)
        o[dst_key_s[sel] // NL_pad, dst_key_s[sel] % NL_pad, k_of[sel]] = \
            gid_s[sel]
        m[dst_key_s[sel] // NL_pad, dst_key_s[sel] % NL_pad, k_of[sel]] = 1.0
        offs.append(o)
        msks.append(m)

    in_maps = []
    wdT = np.ascontiguousarray(W_dense.T)            # [IN, EMB]
    wdT_packed = np.concatenate(
        [wdT[k * P:(k + 1) * P, :] for k in range(cfg.IN // P)], axis=1)
    att = np.stack([att_src, att_dst], axis=1)       # [OUT, 2]
    KC = cfg.IN // P
    for r in range(R):
        xp = np.zeros((NL_pad, cfg.IN), dtype=np.float32)
        xp[:NL] = x[r * NL + orders[r]]
        # transposed, tiled: block (t, k) = xp[tP:(t+1)P, kP:(k+1)P].T
        xT = np.empty((P, TILES * KC * P), dtype=bf16)
        for t in range(TILES):
            blk = xp[t * P:(t + 1) * P, :].T.astype(bf16)   # [IN, P]
            xT[:, (t * KC) * P:(t + 1) * KC * P] = \
                blk.reshape(KC, P, P).transpose(1, 0, 2).reshape(P, KC * P)
        offs_w, msks_w = [], []
        for h in range(2):
            cols, mcols = [], []
            for t in range(TILES):
                Dt = int(D_lists[h][t])
                lin = offs[h][r, t * P:(t + 1) * P, :Dt]     # [P, Dt]
                cols.append(_wrap_idx(lin.T.reshape(-1)))
                mcols.append(msks[h][r, t * P:(t + 1) * P, :Dt])
            offs_w.append(np.ascontiguousarray(np.concatenate(cols, axis=1)))
            msks_w.append(np.ascontiguousarray(
                np.concatenate(mcols, axis=1).astype(bf16)))
        in_maps.append({
            "xT": xT,
            "wdT": wdT_packed.astype(bf16),
            "bd": b_dense.reshape(cfg.EMB, 1).astype(np.float32),
            "wgT": np.ascontiguousarray(W_gat.T).astype(bf16),
            "att": att.astype(bf16),
            "bgat": b_gat.reshape(cfg.OUT, 1).astype(np.float32),
            "offsA": offs_w[0],
            "offsB": offs_w[1],
            "maskA": msks_w[0],
            "maskB": msks_w[1],
        })
    return in_maps, orders, D_lists


def _assemble(cfg, results, orders):
    out = np.empty((cfg.N, cfg.OUT), dtype=np.float32)
    for r in range(R):
        o = results[r]["out"][:cfg.NL]
        out[r * cfg.NL + orders[r]] = o
    return out


# --------------------------------------------------------------------------
# device graph
# --------------------------------------------------------------------------

def _build_graph(cfg, D_lists):
    import concourse.bass as bass
    import concourse.bacc as bacc
    import concourse.mybir as mybir
    import concourse.tile as tile
    from concourse.masks import make_identity

    IN, EMB, OUT = cfg.IN, cfg.EMB, cfg.OUT
    KC = IN // P
    TILES, NL_pad, NHALF = cfg.TILES, cfg.NL_pad, cfg.NHALF
    H, HTILES = cfg.H, cfg.HTILES
    TOT = [int(d.sum()) for d in D_lists]
    fp32 = mybir.dt.float32
    b16 = mybir.dt.bfloat16
    i16 = mybir.dt.int16

    nc = bacc.Bacc(None, target_bir_lowering=False, debug=False, num_devices=R,
                   num_swdge_queues=NQ)

    xT = nc.dram_tensor("xT", [P, TILES * KC * P], b16, kind="ExternalInput")
    wdT = nc.dram_tensor("wdT", [P, KC * EMB], b16, kind="ExternalInput")
    bd = nc.dram_tensor("bd", [EMB, 1], fp32, kind="ExternalInput")
    wgT = nc.dram_tensor("wgT", [EMB, OUT], b16, kind="ExternalInput")
    att = nc.dram_tensor("att", [OUT, 2], b16, kind="ExternalInput")
    bgat = nc.dram_tensor("bgat", [OUT, 1], fp32, kind="ExternalInput")
    offs_ext = [
        nc.dram_tensor("offsA", [P, 8 * TOT[0]], i16, kind="ExternalInput"),
        nc.dram_tensor("offsB", [P, 8 * TOT[1]], i16, kind="ExternalInput"),
    ]
    mask_ext = [
        nc.dram_tensor("maskA", [P, TOT[0]], b16, kind="ExternalInput"),
        nc.dram_tensor("maskB", [P, TOT[1]], b16, kind="ExternalInput"),
    ]
    out = nc.dram_tensor("out", [NL_pad, OUT], fp32, kind="ExternalOutput")

    with tile.TileContext(nc) as tc:
        with (
            tc.tile_pool(name="dram", bufs=1, space="DRAM") as dram,
            tc.tile_pool(name="const", bufs=1) as cst,
        ):
            shards = [dram.tile([H, W_ROW], b16, name=f"shard{i}")
                      for i in range(2)]
            tables = [dram.tile([NHALF, W_ROW], b16, addr_space="Shared",
                                name=f"table{i}")
                      for i in range(2)]

            identb = cst.tile([P, P], b16)
            make_identity(nc, identb[:])
            identf = cst.tile([P, P], fp32)
            make_identity(nc, identf[:])

            wdTs = cst.tile([P, KC * EMB], b16)
            nc.sync.dma_start(out=wdTs[:], in_=wdT[:, :])
            bds = cst.tile([EMB, 1], fp32)
            nc.sync.dma_start(out=bds[:], in_=bd[:, :])
            wgTs = cst.tile([EMB, OUT], b16)
            nc.sync.dma_start(out=wgTs[:], in_=wgT[:, :])
            atts = cst.tile([OUT, 2], b16)
            nc.sync.dma_start(out=atts[:], in_=att[:, :])
            bgs = cst.tile([OUT, 1], fp32)
            nc.sync.dma_start(out=bgs[:], in_=bgat[:, :])
            adst_all = cst.tile([P, TILES], fp32)
            adst2_all = cst.tile([P, TILES], fp32)

            # preload gather indices + pad masks (pure inputs)
            oidx = [cst.tile([P, 8 * TOT[h]], i16, name=f"oidx{h}")
                    for h in range(2)]
            masks = [cst.tile([P, TOT[h]], b16, name=f"mask{h}")
                     for h in range(2)]
            for h in range(2):
                nc.sync.dma_start(out=oidx[h][:], in_=offs_ext[h][:, :])
                nc.sync.dma_start(out=masks[h][:], in_=mask_ext[h][:, :])

            shard_f32 = [s[:].bitcast(fp32) for s in shards]  # [H, 64]

            # ---------------- fused node + edge phases ----------------
            with (
                tc.tile_pool(name="npsum_h", bufs=2, space="PSUM") as ps_h,
                tc.tile_pool(name="npsum_m", bufs=1, space="PSUM") as ps_m,
                tc.tile_pool(name="nsb", bufs=4) as nsb,
                tc.tile_pool(name="acc", bufs=1) as accp,
                tc.tile_pool(name="egp", bufs=6) as egp,
                tc.tile_pool(name="esb", bufs=3) as esb,
            ):
                bgp = ps_m.tile([P, OUT], fp32, tag="misc")
                nc.tensor.transpose(out=bgp[:], in_=bgs[:].to_broadcast([OUT, P]),
                                    identity=identf[:OUT, :OUT])
                bgmat = cst.tile([P, OUT], fp32)
                nc.vector.tensor_copy(bgmat[:], bgp[:])

                numA = [accp.tile([P, OUT], fp32, name=f"numA{t}")
                        for t in range(TILES)]
                denA = [accp.tile([P, 1], fp32, name=f"denA{t}")
                        for t in range(TILES)]
                cum = [0, 0]
                qstate = [0]

                def node_tile(t):
                    hh = 0 if t < HTILES else 1
                    row0 = t * P - hh * H
                    xTs = nsb.tile([P, KC * P], b16, tag="xTs", name="xTs")
                    nc.sync.dma_start(
                        out=xTs[:], in_=xT[:, t * KC * P:(t + 1) * KC * P])
                    hTp = ps_h.tile([EMB, P], fp32, tag="hT", name="hTp")
                    for k in range(KC):
                        nc.tensor.matmul(out=hTp[:],
                                         lhsT=wdTs[:, k * EMB:(k + 1) * EMB],
                                         rhs=xTs[:, k * P:(k + 1) * P],
                                         start=(k == 0), stop=(k == KC - 1))
                    u = nsb.tile([EMB, P], fp32, tag="u", name="u")
                    nc.scalar.activation(u[:], hTp[:],
                                         mybir.ActivationFunctionType.Identity,
                                         bias=bds[:, :1])
                    hT = nsb.tile([EMB, P], b16, tag="hT_sb", name="hT")
                    nc.vector.scalar_tensor_tensor(
                        out=hT[:], in0=u[:], scalar=0.01, in1=u[:],
                        op0=mybir.AluOpType.mult, op1=mybir.AluOpType.max)
                    gTp = ps_m.tile([OUT, P], fp32, tag="misc", name="gTp")
                    nc.tensor.matmul(out=gTp[:], lhsT=wgTs[:], rhs=hT[:],
                                     start=True, stop=True)
                    stg = nsb.tile([OUT, P], b16, tag="stg", name="stg")
                    nc.vector.tensor_copy(stg[:], gTp[:])
                    app = ps_m.tile([2, P], fp32, tag="app", name="app")
                    nc.tensor.matmul(out=app[:], lhsT=atts[:], rhs=stg[:],
                                     start=True, stop=True)
                    ttp = ps_m.tile([P, OUT], b16, tag="ttp", name="ttp")
                    nc.tensor.transpose(out=ttp[:], in_=stg[:],
                                        identity=identb[:OUT, :OUT])
                    tabs = nsb.tile([P, OUT], b16, tag="tabs", name="tabs")
                    nc.scalar.activation(tabs[:], ttp[:],
                                         mybir.ActivationFunctionType.Copy)
                    nc.sync.dma_start(
                        out=shards[hh][row0:row0 + P, 0:OUT], in_=tabs[:])
                    aps = nsb.tile([2, P], fp32, tag="aps", name="aps")
                    nc.vector.tensor_copy(aps[:], app[:])
                    atp = ps_m.tile([P, 2], fp32, tag="atp", name="atp")
                    nc.tensor.transpose(out=atp[:], in_=aps[:],
                                        identity=identf[:2, :2])
                    aTs = nsb.tile([P, 2], fp32, tag="aTs", name="aTs")
                    nc.vector.tensor_copy(aTs[:], atp[:])
                    nc.vector.tensor_copy(adst_all[:, t:t + 1], aTs[:, 1:2])
                    nc.scalar.activation(adst2_all[:, t:t + 1], aTs[:, 1:2],
                                         mybir.ActivationFunctionType.Copy,
                                         scale=0.2)
                    nc.sync.dma_start(
                        out=shard_f32[hh][row0:row0 + P,
                                          A_SRC_F32:A_SRC_F32 + 2],
                        in_=aTs[:])

                def sweep_tile(h, t):
                    D = int(D_lists[h][t])
                    adst = adst_all[:, t:t + 1]
                    adst02 = adst2_all[:, t:t + 1]
                    gpad = egp.tile([P, D * W_ROW], b16, tag="gp", name="gpad")
                    for j in range(0, D, DCALL):
                        Dj = min(DCALL, D - j)
                        nc.gpsimd.dma_gather(
                            out_ap=gpad[:, j * W_ROW:(j + Dj) * W_ROW]
                                .rearrange("p (d w) -> p d w", w=W_ROW),
                            in_ap=tables[h][:, :],
                            idxs_ap=oidx[h][:, 8 * (cum[h] + j):
                                            8 * (cum[h] + j + Dj)],
                            num_idxs=P * Dj, num_idxs_reg=P * Dj,
                            elem_size=W_ROW,
                            queue_num=qstate[0] % NQ,
                        )
                        qstate[0] += 1
                    asrc = gpad[:].bitcast(fp32).rearrange(
                        "p (d w) -> p d w", w=W_ROW // 2)[:, :, A_SRC_F32]
                    t1 = esb.tile([P, D], fp32, tag="t1", name="t1")
                    nc.scalar.activation(t1[:], asrc,
                                         mybir.ActivationFunctionType.Exp,
                                         bias=adst, scale=1.0)
                    t2 = esb.tile([P, D], fp32, tag="t2", name="t2")
                    nc.scalar.activation(t2[:], asrc,
                                         mybir.ActivationFunctionType.Exp,
                                         bias=adst02, scale=0.2)
                    Traw = esb.tile([P, D], fp32, tag="Traw", name="Traw")
                    nc.vector.tensor_tensor(out=Traw[:], in0=t1[:], in1=t2[:],
                                            op=mybir.AluOpType.max)
                    Tm = esb.tile([P, D], b16, tag="Tm", name="Tm")
                    nc.vector.tensor_tensor(out=Tm[:], in0=Traw[:],
                                            in1=masks[h][:, cum[h]:cum[h] + D],
                                            op=mybir.AluOpType.mult)
                    den = denA[t] if h == 0 else esb.tile([P, 1], fp32,
                                                          tag="den", name="den")
                    nc.vector.tensor_reduce(out=den[:], in_=Tm[:],
                                            op=mybir.AluOpType.add,
                                            axis=mybir.AxisListType.X)
                    gsc = esb.tile([P, D * OUT], b16, tag="gsc", name="gsc")
                    nc.vector.tensor_tensor(
                        out=gsc[:].rearrange("p (d c) -> p d c", c=OUT),
                        in0=gpad[:].rearrange("p (d w) -> p d w",
                                              w=W_ROW)[:, :, 0:OUT],
                        in1=Tm[:].to_broadcast([P, D, OUT]),
                        op=mybir.AluOpType.mult)
                    num = numA[t] if h == 0 else esb.tile([P, OUT], fp32,
                                                          tag="num", name="num")
                    nc.vector.tensor_reduce(
                        out=num[:],
                        in_=gsc[:].rearrange("p (d c) -> p c d", c=OUT),
                        op=mybir.AluOpType.add, axis=mybir.AxisListType.X)
                    if h == 1:
                        stot = esb.tile([P, OUT], fp32, tag="stot", name="stot")
                        nc.vector.tensor_tensor(out=stot[:], in0=num[:],
                                                in1=numA[t][:],
                                                op=mybir.AluOpType.add)
                        dtot = esb.tile([P, 1], fp32, tag="dtot", name="dtot")
                        nc.vector.tensor_tensor(out=dtot[:], in0=den[:],
                                                in1=denA[t][:],
                                                op=mybir.AluOpType.add)
                        rden = esb.tile([P, 1], fp32, tag="rden", name="rden")
                        nc.vector.reciprocal(rden[:], dtot[:])
                        outf = esb.tile([P, OUT], fp32, tag="outf", name="outf")
                        nc.vector.scalar_tensor_tensor(
                            out=outf[:], in0=stot[:], scalar=rden[:, :1],
                            in1=bgmat[:],
                            op0=mybir.AluOpType.mult,
                            op1=mybir.AluOpType.add)
                        nc.sync.dma_start(out=out[t * P:(t + 1) * P, :],
                                          in_=outf[:])
                    cum[h] += D

                # schedule: node 0..H-1, AG1, then interleave the second half
                # of the node phase with early sweep-A tiles (Pool is idle
                # during node tiles), AG2, remaining sweep A, sweep B.
                for t in range(HTILES):
                    node_tile(t)
                nc.gpsimd.collective_compute(
                    "AllGather", mybir.AluOpType.bypass,
                    replica_groups=[list(range(R))],
                    ins=[shards[0].opt()], outs=[tables[0].opt()],
                )
                for t in range(HTILES, TILES):
                    node_tile(t)
                nc.gpsimd.collective_compute(
                    "AllGather", mybir.AluOpType.bypass,
                    replica_groups=[list(range(R))],
                    ins=[shards[1].opt()], outs=[tables[1].opt()],
                )
                for t in range(TILES):
                    sweep_tile(0, t)
                for t in range(TILES):
                    sweep_tile(1, t)
    nc.finalize()
    return nc


# --------------------------------------------------------------------------
# entry points
# --------------------------------------------------------------------------

def run(inputs, cfg=CFG_REAL, trace=False):
    from concourse.bass_utils import run_bass_kernel_spmd
    in_maps, orders, D_lists = _prepare(cfg, **inputs)
    nc = _build_graph(cfg, D_lists)
    res = run_bass_kernel_spmd(nc, in_maps, core_ids=list(range(R)),
                               trace=trace)
    out = _assemble(cfg, res.results, orders)
    return out, res


def kernel(**inputs):
    inputs = {k: np.asarray(v) for k, v in inputs.items()}
    out, _ = run(inputs, CFG_REAL, trace=False)
    return out


# revision 16
# speedup vs baseline: 1.2165x; 1.2165x over previous
"""Distributed GAT (AnomalyDAE encoder) kernel for 8 TRN2 NeuronCores.

Reference computation:
    h = leaky_relu(x @ W_dense.T + b_dense, 0.01)          # [N, 128]
    g = h @ W_gat.T                                        # [N, 64]
    a_src = g @ att_src ; a_dst = g @ att_dst              # [N]
    with self-loops appended, per edge (s -> d):
        e = leaky_relu(a_src[s] + a_dst[d], 0.2)
        alpha = segment_softmax(e, by d)
    out[d] = sum_e alpha_e * g[s_e] + b_gat                # [N, 64]

Sharding: nodes split contiguously across 8 cores (6250 each); edges
partitioned by destination core. Each core's local nodes are sorted by
in-degree on the host so 128-node tiles have near-uniform degree; per-tile
edge lists are padded to the tile max degree with host-built 0/1 masks
(replacing sentinel rows).

Device pipeline per core:
  node phase: per 128-node tile, matmuls against host-pretransposed x
    -> hT -> lrelu -> gT -> a_src/a_dst matvec -> PE-transpose -> table rows
    [g bf16(64) | a_src f32 | a_dst f32 | pad] (256 B) into one of two DRAM
    shard halves split by local position (pos < 3200 -> A).
  Two AllGathers (one per half): AG1 fires after node tiles 0-24 and
    overlaps the rest of the node phase; AG2 after tile 49.
  edge phase: two sweeps (A then B). Each sweep streams gpsimd dma_gather
    calls back-to-back over rotating SWDGE queues (Pool engine saturated),
    while DVE/ACT trail behind computing exp(LRelu) weights, the pad mask,
    and partial weighted sums into per-tile accumulators [P,64]+[P,1].
  Sweep B finalizes: out = (numA+numB) / (denA+denB) + b_gat.

Softmax is computed without the segment-max shift (logits are O(1); result
identical in exact arithmetic, and the two half partial sums compose
exactly).
"""

import numpy as np
import ml_dtypes

bf16 = ml_dtypes.bfloat16

R = 8            # cores
P = 128          # partitions / tile size
W_ROW = 128      # table row width in bf16 elems (256 B rows for dma_gather)
A_SRC_F32 = 32   # f32 column of a_src within a row (byte offset 128)
DCALL = 8        # gather rows per dma_gather call chunk (1024-desc ring)
NQ = 4           # SWDGE queues


class Cfg:
    def __init__(self, N, E, IN=512, EMB=128, OUT=64):
        assert N % R == 0
        self.N, self.E, self.IN, self.EMB, self.OUT = N, E, IN, EMB, OUT
        self.NL = N // R
        nlp = ((self.NL + 2 * P - 1) // (2 * P)) * (2 * P)
        self.NL_pad = nlp
        self.TILES = nlp // P
        self.H = nlp // 2                    # local-position half split
        self.HTILES = self.H // P
        self.NHALF = self.H * R              # rows per table half
        assert self.NHALF < 32768, "dma_gather int16 index limit"


CFG_REAL = Cfg(N=50000, E=1600000)


# --------------------------------------------------------------------------
# host-side preprocessing
# --------------------------------------------------------------------------

def _wrap_idx(lin):
    """dma_gather index layout: linear i -> [i % 16, i // 16], replicated
    across the 8 Q7 core groups -> [128, len/16] int16."""
    assert len(lin) % 16 == 0
    w = lin.reshape(-1, 16).T.astype(np.int16)
    return np.tile(w, (8, 1))


def _prepare(cfg, x, edge_index, W_dense, b_dense, W_gat, att_src, att_dst,
             b_gat):
    N, NL, NL_pad, TILES = cfg.N, cfg.NL, cfg.NL_pad, cfg.TILES
    src = edge_index[0].astype(np.int64)
    dst = edge_index[1].astype(np.int64)
    # self-loops are handled on-chip from the local g tile, not gathered
    deg = np.bincount(dst, minlength=N)

    pos_of = np.empty(N, dtype=np.int64)
    orders = []
    for r in range(R):
        dloc = deg[r * NL:(r + 1) * NL]
        order = np.argsort(-dloc, kind="stable")
        orders.append(order)
        pos_of[r * NL + order] = np.arange(NL)

    core_of_node = np.arange(N) // NL
    pos_src = pos_of[src]
    in_b = pos_src >= cfg.H                  # source in half B (by position)
    gid_half = core_of_node[src] * cfg.H + np.where(in_b, pos_src - cfg.H,
                                                    pos_src)

    # per-(core,position,half) degrees
    degh = np.zeros((2, R * NL_pad), dtype=np.int64)
    dst_key = core_of_node[dst] * NL_pad + pos_of[dst]
    np.add.at(degh[0], dst_key[~in_b], 1)
    np.add.at(degh[1], dst_key[in_b], 1)
    D_lists = []
    for h in range(2):
        dt_ = degh[h].reshape(R, TILES, P).max(axis=(0, 2))
        D_lists.append(np.maximum(dt_, 1).astype(np.int64))

    # sort edges by (dst position, half, gid)
    ekey = dst_key * 2 + in_b
    eorder = np.lexsort((gid_half, ekey))
    dst_key_s = dst_key[eorder]
    hb_s = in_b[eorder]
    gid_s = gid_half[eorder]
    # slot index within the (dst, half) group
    grp = dst_key_s * 2 + hb_s
    starts = np.zeros(2 * R * NL_pad + 1, dtype=np.int64)
    np.add.at(starts, grp + 1, 1)
    starts = np.cumsum(starts)
    k_of = np.arange(len(gid_s)) - starts[grp]

    # offs[h]: int32 [R, NL_pad, Dmax_h]; pad slots are masked out by msks[h].
    # Pad targets are SCATTERED across the table: a single shared pad row
    # makes whole gather calls hammer one 256-B row -> HBM channel hotspot
    # that serializes all 16 SDMA engines (~10x slowdown on those calls).
    offs, msks = [], []
    for h in range(2):
        Dmax = int(D_lists[h].max())
        o = ((np.arange(R)[:, None, None] * 3181
              + np.arange(NL_pad)[None, :, None] * 131
              + np.arange(Dmax)[None, None, :] * 7919) % cfg.NHALF
             ).astype(np.int32)
        m = np.zeros((R, NL_pad, Dmax), dtype=np.float32)
        sel = hb_s == bool(h)
        o[dst_key_s[sel] // NL_pad, dst_key_s[sel] % NL_pad, k_of[sel]] = \
            gid_s[sel]
        m[dst_key_s[sel] // NL_pad, dst_key_s[sel] % NL_pad, k_of[sel]] = 1.0
        offs.append(o)
        msks.append(m)

    in_maps = []
    wdT = np.ascontiguousarray(W_dense.T)            # [IN, EMB]
    wdT_packed = np.concatenate(
        [wdT[k * P:(k + 1) * P, :] for k in range(cfg.IN // P)], axis=1)
    att = np.stack([att_src, att_dst], axis=1)       # [OUT, 2]
    KC = cfg.IN // P
    for r in range(R):
        xp = np.zeros((NL_pad, cfg.IN), dtype=np.float32)
        xp[:NL] = x[r * NL + orders[r]]
        # transposed, tiled: block (t, k) = xp[tP:(t+1)P, kP:(k+1)P].T
        xT = np.empty((P, TILES * KC * P), dtype=bf16)
        for t in range(TILES):
            blk = xp[t * P:(t + 1) * P, :].T.astype(bf16)   # [IN, P]
            xT[:, (t * KC) * P:(t + 1) * KC * P] = \
                blk.reshape(KC, P, P).transpose(1, 0, 2).reshape(P, KC * P)
        offs_w, msks_w = [], []
        for h in range(2):
            cols, mcols = [], []
            for t in range(TILES):
                Dt = int(D_lists[h][t])
                lin = offs[h][r, t * P:(t + 1) * P, :Dt]     # [P, Dt]
                cols.append(_wrap_idx(lin.T.reshape(-1)))
                mcols.append(msks[h][r, t * P:(t + 1) * P, :Dt])
            offs_w.append(np.ascontiguousarray(np.concatenate(cols, axis=1)))
            msks_w.append(np.ascontiguousarray(
                np.concatenate(mcols, axis=1).astype(bf16)))
        in_maps.append({
            "xT": xT,
            "wdT": wdT_packed.astype(bf16),
            "bd": b_dense.reshape(cfg.EMB, 1).astype(np.float32),
            "wgT": np.ascontiguousarray(W_gat.T).astype(bf16),
            "att": att.astype(bf16),
            "bgat": b_gat.reshape(cfg.OUT, 1).astype(np.float32),
            "offsA": offs_w[0],
            "offsB": offs_w[1],
            "maskA": msks_w[0],
            "maskB": msks_w[1],
        })
    return in_maps, orders, D_lists


def _assemble(cfg, results, orders):
    out = np.empty((cfg.N, cfg.OUT), dtype=np.float32)
    for r in range(R):
        o = results[r]["out"][:cfg.NL]
        out[r * cfg.NL + orders[r]] = o
    return out


# --------------------------------------------------------------------------
# device graph
# --------------------------------------------------------------------------

def _build_graph(cfg, D_lists):
    import concourse.bass as bass
    import concourse.bacc as bacc
    import concourse.mybir as mybir
    import concourse.tile as tile
    from concourse.masks import make_identity

    IN, EMB, OUT = cfg.IN, cfg.EMB, cfg.OUT
    KC = IN // P
    TILES, NL_pad, NHALF = cfg.TILES, cfg.NL_pad, cfg.NHALF
    H, HTILES = cfg.H, cfg.HTILES
    TOT = [int(d.sum()) for d in D_lists]
    fp32 = mybir.dt.float32
    b16 = mybir.dt.bfloat16
    i16 = mybir.dt.int16

    nc = bacc.Bacc(None, target_bir_lowering=False, debug=False, num_devices=R,
                   num_swdge_queues=NQ)

    xT = nc.dram_tensor("xT", [P, TILES * KC * P], b16, kind="ExternalInput")
    wdT = nc.dram_tensor("wdT", [P, KC * EMB], b16, kind="ExternalInput")
    bd = nc.dram_tensor("bd", [EMB, 1], fp32, kind="ExternalInput")
    wgT = nc.dram_tensor("wgT", [EMB, OUT], b16, kind="ExternalInput")
    att = nc.dram_tensor("att", [OUT, 2], b16, kind="ExternalInput")
    bgat = nc.dram_tensor("bgat", [OUT, 1], fp32, kind="ExternalInput")
    offs_ext = [
        nc.dram_tensor("offsA", [P, 8 * TOT[0]], i16, kind="ExternalInput"),
        nc.dram_tensor("offsB", [P, 8 * TOT[1]], i16, kind="ExternalInput"),
    ]
    mask_ext = [
        nc.dram_tensor("maskA", [P, TOT[0]], b16, kind="ExternalInput"),
        nc.dram_tensor("maskB", [P, TOT[1]], b16, kind="ExternalInput"),
    ]
    out = nc.dram_tensor("out", [NL_pad, OUT], fp32, kind="ExternalOutput")

    with tile.TileContext(nc) as tc:
        with (
            tc.tile_pool(name="dram", bufs=1, space="DRAM") as dram,
            tc.tile_pool(name="const", bufs=1) as cst,
        ):
            shards = [dram.tile([H, W_ROW], b16, name=f"shard{i}")
                      for i in range(2)]
            tables = [dram.tile([NHALF, W_ROW], b16, addr_space="Shared",
                                name=f"table{i}")
                      for i in range(2)]

            identb = cst.tile([P, P], b16)
            make_identity(nc, identb[:])
            identf = cst.tile([P, P], fp32)
            make_identity(nc, identf[:])

            wdTs = cst.tile([P, KC * EMB], b16)
            nc.sync.dma_start(out=wdTs[:], in_=wdT[:, :])
            bds = cst.tile([EMB, 1], fp32)
            nc.sync.dma_start(out=bds[:], in_=bd[:, :])
            wgTs = cst.tile([EMB, OUT], b16)
            nc.sync.dma_start(out=wgTs[:], in_=wgT[:, :])
            atts = cst.tile([OUT, 2], b16)
            nc.sync.dma_start(out=atts[:], in_=att[:, :])
            bgs = cst.tile([OUT, 1], fp32)
            nc.sync.dma_start(out=bgs[:], in_=bgat[:, :])
            adst_all = cst.tile([P, TILES], fp32)
            adst2_all = cst.tile([P, TILES], fp32)

            # preload gather indices + pad masks (pure inputs)
            oidx = [cst.tile([P, 8 * TOT[h]], i16, name=f"oidx{h}")
                    for h in range(2)]
            masks = [cst.tile([P, TOT[h]], b16, name=f"mask{h}")
                     for h in range(2)]
            for h in range(2):
                nc.sync.dma_start(out=oidx[h][:], in_=offs_ext[h][:, :])
                nc.sync.dma_start(out=masks[h][:], in_=mask_ext[h][:, :])

            shard_f32 = [s[:].bitcast(fp32) for s in shards]  # [H, 64]

            # ---------------- node phase ----------------
            gloc = cst.tile([P, TILES * OUT], b16)
            asrc_all = cst.tile([P, TILES], fp32)
            with (
                tc.tile_pool(name="npsum_h", bufs=2, space="PSUM") as ps_h,
                tc.tile_pool(name="npsum_m", bufs=1, space="PSUM") as ps_m,
                tc.tile_pool(name="nsb", bufs=4) as nsb,
            ):
                bgp = ps_m.tile([P, OUT], fp32, tag="misc")
                nc.tensor.transpose(out=bgp[:], in_=bgs[:].to_broadcast([OUT, P]),
                                    identity=identf[:OUT, :OUT])
                bgmat = cst.tile([P, OUT], fp32)
                nc.vector.tensor_copy(bgmat[:], bgp[:])

                for t in range(TILES):
                    hh = 0 if t < HTILES else 1
                    row0 = t * P - hh * H
                    xTs = nsb.tile([P, KC * P], b16, tag="xTs")
                    nc.sync.dma_start(
                        out=xTs[:], in_=xT[:, t * KC * P:(t + 1) * KC * P])
                    hTp = ps_h.tile([EMB, P], fp32, tag="hT")
                    for k in range(KC):
                        nc.tensor.matmul(out=hTp[:],
                                         lhsT=wdTs[:, k * EMB:(k + 1) * EMB],
                                         rhs=xTs[:, k * P:(k + 1) * P],
                                         start=(k == 0), stop=(k == KC - 1))
                    hT = nsb.tile([EMB, P], b16, tag="hT_sb")
                    nc.scalar.activation(hT[:], hTp[:],
                                         mybir.ActivationFunctionType.Lrelu,
                                         bias=bds[:, :1])
                    gTp = ps_m.tile([OUT, P], fp32, tag="misc")
                    nc.tensor.matmul(out=gTp[:], lhsT=wgTs[:], rhs=hT[:],
                                     start=True, stop=True)
                    stg = nsb.tile([OUT, P], b16, tag="stg")
                    nc.vector.tensor_copy(stg[:], gTp[:])
                    app = ps_m.tile([2, P], fp32, tag="app")
                    nc.tensor.matmul(out=app[:], lhsT=atts[:], rhs=stg[:],
                                     start=True, stop=True)
                    # transpose gT -> table g block
                    ttp = ps_m.tile([P, OUT], b16, tag="ttp")
                    nc.tensor.transpose(out=ttp[:], in_=stg[:],
                                        identity=identb[:OUT, :OUT])
                    tabs = nsb.tile([P, OUT], b16, tag="tabs")
                    nc.scalar.activation(tabs[:], ttp[:],
                                         mybir.ActivationFunctionType.Copy)
                    nc.vector.tensor_copy(gloc[:, t * OUT:(t + 1) * OUT],
                                          tabs[:])
                    nc.sync.dma_start(
                        out=shards[hh][row0:row0 + P, 0:OUT], in_=tabs[:])
                    # transpose [a_src; a_dst] -> [P, 2] f32
                    aps = nsb.tile([2, P], fp32, tag="aps")
                    nc.vector.tensor_copy(aps[:], app[:])
                    atp = ps_m.tile([P, 2], fp32, tag="atp")
                    nc.tensor.transpose(out=atp[:], in_=aps[:],
                                        identity=identf[:2, :2])
                    aTs = nsb.tile([P, 2], fp32, tag="aTs")
                    nc.vector.tensor_copy(aTs[:], atp[:])
                    nc.vector.tensor_copy(adst_all[:, t:t + 1], aTs[:, 1:2])
                    nc.vector.tensor_copy(asrc_all[:, t:t + 1], aTs[:, 0:1])
                    nc.sync.dma_start(
                        out=shard_f32[hh][row0:row0 + P,
                                          A_SRC_F32:A_SRC_F32 + 2],
                        in_=aTs[:])
                    if t == HTILES - 1:
                        nc.gpsimd.collective_compute(
                            "AllGather", mybir.AluOpType.bypass,
                            replica_groups=[list(range(R))],
                            ins=[shards[0].opt()], outs=[tables[0].opt()],
                        )

                nc.scalar.activation(adst2_all[:], adst_all[:],
                                     mybir.ActivationFunctionType.Copy,
                                     scale=0.2)

            # ---------------- edge phase: two sweeps ----------------
            qi = 0
            with (
                tc.tile_pool(name="acc", bufs=1) as accp,
                tc.tile_pool(name="egp", bufs=6) as egp,
                tc.tile_pool(name="esb", bufs=3) as esb,
            ):
                numA = [accp.tile([P, OUT], fp32, name=f"numA{t}")
                        for t in range(TILES)]
                denA = [accp.tile([P, 1], fp32, name=f"denA{t}")
                        for t in range(TILES)]

                first_sweep = True
                for h in range(2):
                    cum = 0
                    for t in range(TILES):
                        if first_sweep:
                            nc.gpsimd.collective_compute(
                                "AllGather", mybir.AluOpType.bypass,
                                replica_groups=[list(range(R))],
                                ins=[shards[1].opt()],
                                outs=[tables[1].opt()],
                            )
                            first_sweep = False
                        D = int(D_lists[h][t])
                        adst = adst_all[:, t:t + 1]
                        adst02 = adst2_all[:, t:t + 1]
                        gpad = egp.tile([P, D * W_ROW], b16, tag="gp")
                        for j in range(0, D, DCALL):
                            Dj = min(DCALL, D - j)
                            nc.gpsimd.dma_gather(
                                out_ap=gpad[:, j * W_ROW:(j + Dj) * W_ROW]
                                    .rearrange("p (d w) -> p d w", w=W_ROW),
                                in_ap=tables[h][:, :],
                                idxs_ap=oidx[h][:, 8 * (cum + j):
                                                8 * (cum + j + Dj)],
                                num_idxs=P * Dj, num_idxs_reg=P * Dj,
                                elem_size=W_ROW,
                                queue_num=qi % NQ,
                            )
                            qi += 1
                        asrc = gpad[:].bitcast(fp32).rearrange(
                            "p (d w) -> p d w", w=W_ROW // 2)[:, :, A_SRC_F32]
                        t1 = esb.tile([P, D], fp32, tag="t1")
                        nc.scalar.activation(t1[:], asrc,
                                             mybir.ActivationFunctionType.Exp,
                                             bias=adst, scale=1.0)
                        t2 = esb.tile([P, D], fp32, tag="t2")
                        nc.scalar.activation(t2[:], asrc,
                                             mybir.ActivationFunctionType.Exp,
                                             bias=adst02, scale=0.2)
                        Traw = esb.tile([P, D], fp32, tag="Traw")
                        nc.vector.tensor_tensor(out=Traw[:], in0=t1[:],
                                                in1=t2[:],
                                                op=mybir.AluOpType.max)
                        # mask pads, then reduce for the denominator partial
                        Tm = esb.tile([P, D], b16, tag="Tm")
                        nc.vector.tensor_tensor(out=Tm[:], in0=Traw[:],
                                                in1=masks[h][:, cum:cum + D],
                                                op=mybir.AluOpType.mult)
                        den = denA[t] if h == 0 else esb.tile([P, 1], fp32,
                                                              tag="den")
                        nc.vector.tensor_reduce(out=den[:], in_=Tm[:],
                                                op=mybir.AluOpType.add,
                                                axis=mybir.AxisListType.X)
                        # weighted rows
                        gsc = esb.tile([P, D * OUT], b16, tag="gsc")
                        nc.vector.tensor_tensor(
                            out=gsc[:].rearrange("p (d c) -> p d c", c=OUT),
                            in0=gpad[:].rearrange("p (d w) -> p d w",
                                                  w=W_ROW)[:, :, 0:OUT],
                            in1=Tm[:].to_broadcast([P, D, OUT]),
                            op=mybir.AluOpType.mult)
                        num = numA[t] if h == 0 else esb.tile([P, OUT], fp32,
                                                              tag="num")
                        nc.vector.tensor_reduce(
                            out=num[:],
                            in_=gsc[:].rearrange("p (d c) -> p c d", c=OUT),
                            op=mybir.AluOpType.add, axis=mybir.AxisListType.X)
                        if h == 1:
                            # self-loop contribution from the local g tile
                            s1 = esb.tile([P, 1], fp32, tag="s1")
                            nc.scalar.activation(
                                s1[:], asrc_all[:, t:t + 1],
                                mybir.ActivationFunctionType.Exp,
                                bias=adst, scale=1.0)
                            s2 = esb.tile([P, 1], fp32, tag="s2")
                            nc.scalar.activation(
                                s2[:], asrc_all[:, t:t + 1],
                                mybir.ActivationFunctionType.Exp,
                                bias=adst02, scale=0.2)
                            wself = esb.tile([P, 1], fp32, tag="wself")
                            nc.vector.tensor_tensor(out=wself[:], in0=s1[:],
                                                    in1=s2[:],
                                                    op=mybir.AluOpType.max)
                            stot = esb.tile([P, OUT], fp32, tag="stot")
                            nc.vector.tensor_tensor(out=stot[:], in0=num[:],
                                                    in1=numA[t][:],
                                                    op=mybir.AluOpType.add)
                            stot2 = esb.tile([P, OUT], fp32, tag="stot2")
                            nc.vector.scalar_tensor_tensor(
                                out=stot2[:],
                                in0=gloc[:, t * OUT:(t + 1) * OUT],
                                scalar=wself[:, :1], in1=stot[:],
                                op0=mybir.AluOpType.mult,
                                op1=mybir.AluOpType.add)
                            dtot = esb.tile([P, 1], fp32, tag="dtot")
                            nc.vector.tensor_tensor(out=dtot[:], in0=den[:],
                                                    in1=denA[t][:],
                                                    op=mybir.AluOpType.add)
                            dtot2 = esb.tile([P, 1], fp32, tag="dtot2")
                            nc.vector.tensor_tensor(out=dtot2[:], in0=dtot[:],
                                                    in1=wself[:],
                                                    op=mybir.AluOpType.add)
                            rden = esb.tile([P, 1], fp32, tag="rden")
                            nc.vector.reciprocal(rden[:], dtot2[:])
                            outf = esb.tile([P, OUT], fp32, tag="outf")
                            nc.vector.scalar_tensor_tensor(
                                out=outf[:], in0=stot2[:], scalar=rden[:, :1],
                                in1=bgmat[:],
                                op0=mybir.AluOpType.mult,
                                op1=mybir.AluOpType.add)
                            nc.sync.dma_start(out=out[t * P:(t + 1) * P, :],
                                              in_=outf[:])
                        cum += D
    nc.finalize()
    return nc


# --------------------------------------------------------------------------
# entry points
# --------------------------------------------------------------------------

def run(inputs, cfg=CFG_REAL, trace=False):
    from concourse.bass_utils import run_bass_kernel_spmd
    in_maps, orders, D_lists = _prepare(cfg, **inputs)
    nc = _build_graph(cfg, D_lists)
    res = run_bass_kernel_spmd(nc, in_maps, core_ids=list(range(R)),
                               trace=trace)
    out = _assemble(cfg, res.results, orders)
    return out, res


def kernel(**inputs):
    inputs = {k: np.asarray(v) for k, v in inputs.items()}
    out, _ = run(inputs, CFG_REAL, trace=False)
    return out


# revision 17
# speedup vs baseline: 1.2516x; 1.0288x over previous
"""Distributed GAT (AnomalyDAE encoder) kernel for 8 TRN2 NeuronCores.

Reference computation:
    h = leaky_relu(x @ W_dense.T + b_dense, 0.01)          # [N, 128]
    g = h @ W_gat.T                                        # [N, 64]
    a_src = g @ att_src ; a_dst = g @ att_dst              # [N]
    with self-loops appended, per edge (s -> d):
        e = leaky_relu(a_src[s] + a_dst[d], 0.2)
        alpha = segment_softmax(e, by d)
    out[d] = sum_e alpha_e * g[s_e] + b_gat                # [N, 64]

Sharding: nodes split contiguously across 8 cores (6250 each); edges
partitioned by destination core. Each core's local nodes are sorted by
in-degree on the host so 128-node tiles have near-uniform degree; per-tile
edge lists are padded to the tile max degree with host-built 0/1 masks
(replacing sentinel rows).

Device pipeline per core:
  node phase: per 128-node tile, matmuls against host-pretransposed x
    -> hT -> lrelu -> gT -> a_src/a_dst matvec -> PE-transpose -> table rows
    [g bf16(64) | a_src f32 | a_dst f32 | pad] (256 B) into one of two DRAM
    shard halves split by local position (pos < 3200 -> A).
  Two AllGathers (one per half): AG1 fires after node tiles 0-24 and
    overlaps the rest of the node phase; AG2 after tile 49.
  edge phase: two sweeps (A then B). Each sweep streams gpsimd dma_gather
    calls back-to-back over rotating SWDGE queues (Pool engine saturated),
    while DVE/ACT trail behind computing exp(LRelu) weights, the pad mask,
    and partial weighted sums into per-tile accumulators [P,64]+[P,1].
  Sweep B finalizes: out = (numA+numB) / (denA+denB) + b_gat.

Softmax is computed without the segment-max shift (logits are O(1); result
identical in exact arithmetic, and the two half partial sums compose
exactly).
"""

import numpy as np
import ml_dtypes

bf16 = ml_dtypes.bfloat16

R = 8            # cores
P = 128          # partitions / tile size
W_ROW = 128      # table row width in bf16 elems (256 B rows for dma_gather)
A_SRC_F32 = 32   # f32 column of a_src within a row (byte offset 128)
DCALL = 8        # gather rows per dma_gather call chunk (1024-desc ring)
NQ = 4           # SWDGE queues


class Cfg:
    def __init__(self, N, E, IN=512, EMB=128, OUT=64):
        assert N % R == 0
        self.N, self.E, self.IN, self.EMB, self.OUT = N, E, IN, EMB, OUT
        self.NL = N // R
        nlp = ((self.NL + 2 * P - 1) // (2 * P)) * (2 * P)
        self.NL_pad = nlp
        self.TILES = nlp // P
        self.H = nlp // 2                    # local-position half split
        self.HTILES = self.H // P
        self.NHALF = self.H * R              # rows per table half
        assert self.NHALF < 32768, "dma_gather int16 index limit"


CFG_REAL = Cfg(N=50000, E=1600000)


# --------------------------------------------------------------------------
# host-side preprocessing
# --------------------------------------------------------------------------

def _wrap_idx(lin):
    """dma_gather index layout: linear i -> [i % 16, i // 16], replicated
    across the 8 Q7 core groups -> [128, len/16] int16."""
    assert len(lin) % 16 == 0
    w = lin.reshape(-1, 16).T.astype(np.int16)
    return np.tile(w, (8, 1))


def _prepare(cfg, x, edge_index, W_dense, b_dense, W_gat, att_src, att_dst,
             b_gat):
    N, NL, NL_pad, TILES = cfg.N, cfg.NL, cfg.NL_pad, cfg.TILES
    src = edge_index[0].astype(np.int64)
    dst = edge_index[1].astype(np.int64)
    # self-loops are handled on-chip from the local g tile, not gathered
    deg = np.bincount(dst, minlength=N)

    pos_of = np.empty(N, dtype=np.int64)
    orders = []
    for r in range(R):
        dloc = deg[r * NL:(r + 1) * NL]
        order = np.argsort(-dloc, kind="stable")
        orders.append(order)
        pos_of[r * NL + order] = np.arange(NL)

    core_of_node = np.arange(N) // NL
    pos_src = pos_of[src]
    in_b = pos_src >= cfg.H                  # source in half B (by position)
    gid_half = core_of_node[src] * cfg.H + np.where(in_b, pos_src - cfg.H,
                                                    pos_src)

    # per-(core,position,half) degrees
    degh = np.zeros((2, R * NL_pad), dtype=np.int64)
    dst_key = core_of_node[dst] * NL_pad + pos_of[dst]
    np.add.at(degh[0], dst_key[~in_b], 1)
    np.add.at(degh[1], dst_key[in_b], 1)
    D_lists = []
    for h in range(2):
        dt_ = degh[h].reshape(R, TILES, P).max(axis=(0, 2))
        D_lists.append(np.maximum(dt_, 1).astype(np.int64))

    # sort edges by (dst position, half, gid)
    ekey = dst_key * 2 + in_b
    eorder = np.lexsort((gid_half, ekey))
    dst_key_s = dst_key[eorder]
    hb_s = in_b[eorder]
    gid_s = gid_half[eorder]
    # slot index within the (dst, half) group
    grp = dst_key_s * 2 + hb_s
    starts = np.zeros(2 * R * NL_pad + 1, dtype=np.int64)
    np.add.at(starts, grp + 1, 1)
    starts = np.cumsum(starts)
    k_of = np.arange(len(gid_s)) - starts[grp]

    # offs[h]: int32 [R, NL_pad, Dmax_h]; pad slots are masked out by msks[h].
    # Pad targets are SCATTERED across the table: a single shared pad row
    # makes whole gather calls hammer one 256-B row -> HBM channel hotspot
    # that serializes all 16 SDMA engines (~10x slowdown on those calls).
    offs, msks = [], []
    for h in range(2):
        Dmax = int(D_lists[h].max())
        o = ((np.arange(R)[:, None, None] * 3181
              + np.arange(NL_pad)[None, :, None] * 131
              + np.arange(Dmax)[None, None, :] * 7919) % cfg.NHALF
             ).astype(np.int32)
        m = np.zeros((R, NL_pad, Dmax), dtype=np.float32)
        sel = hb_s == bool(h)
        o[dst_key_s[sel] // NL_pad, dst_key_s[sel] % NL_pad, k_of[sel]] = \
            gid_s[sel]
        m[dst_key_s[sel] // NL_pad, dst_key_s[sel] % NL_pad, k_of[sel]] = 1.0
        offs.append(o)
        msks.append(m)

    in_maps = []
    wdT = np.ascontiguousarray(W_dense.T)            # [IN, EMB]
    wdT_packed = np.concatenate(
        [wdT[k * P:(k + 1) * P, :] for k in range(cfg.IN // P)], axis=1)
    att = np.stack([att_src, att_dst], axis=1)       # [OUT, 2]
    KC = cfg.IN // P
    for r in range(R):
        xp = np.zeros((NL_pad, cfg.IN), dtype=np.float32)
        xp[:NL] = x[r * NL + orders[r]]
        # transposed, tiled: block (t, k) = xp[tP:(t+1)P, kP:(k+1)P].T
        xT = np.empty((P, TILES * KC * P), dtype=bf16)
        for t in range(TILES):
            blk = xp[t * P:(t + 1) * P, :].T.astype(bf16)   # [IN, P]
            xT[:, (t * KC) * P:(t + 1) * KC * P] = \
                blk.reshape(KC, P, P).transpose(1, 0, 2).reshape(P, KC * P)
        offs_w, msks_w = [], []
        for h in range(2):
            cols, mcols = [], []
            for t in range(TILES):
                Dt = int(D_lists[h][t])
                lin = offs[h][r, t * P:(t + 1) * P, :Dt]     # [P, Dt]
                cols.append(_wrap_idx(lin.T.reshape(-1)))
                mcols.append(msks[h][r, t * P:(t + 1) * P, :Dt])
            offs_w.append(np.ascontiguousarray(np.concatenate(cols, axis=1)))
            msks_w.append(np.ascontiguousarray(
                np.concatenate(mcols, axis=1).astype(bf16)))
        in_maps.append({
            "xT": xT,
            "wdT": wdT_packed.astype(bf16),
            "bd": b_dense.reshape(cfg.EMB, 1).astype(np.float32),
            "wgT": np.ascontiguousarray(W_gat.T).astype(bf16),
            "att": att.astype(bf16),
            "bgat": b_gat.reshape(cfg.OUT, 1).astype(np.float32),
            "offsA": offs_w[0],
            "offsB": offs_w[1],
            "maskA": msks_w[0],
            "maskB": msks_w[1],
        })
    return in_maps, orders, D_lists


def _assemble(cfg, results, orders):
    out = np.empty((cfg.N, cfg.OUT), dtype=np.float32)
    for r in range(R):
        o = results[r]["out"][:cfg.NL]
        out[r * cfg.NL + orders[r]] = o
    return out


# --------------------------------------------------------------------------
# device graph
# --------------------------------------------------------------------------

def _build_graph(cfg, D_lists):
    import concourse.bass as bass
    import concourse.bacc as bacc
    import concourse.mybir as mybir
    import concourse.tile as tile
    from concourse.masks import make_identity

    IN, EMB, OUT = cfg.IN, cfg.EMB, cfg.OUT
    KC = IN // P
    TILES, NL_pad, NHALF = cfg.TILES, cfg.NL_pad, cfg.NHALF
    H, HTILES = cfg.H, cfg.HTILES
    TOT = [int(d.sum()) for d in D_lists]
    fp32 = mybir.dt.float32
    b16 = mybir.dt.bfloat16
    i16 = mybir.dt.int16

    nc = bacc.Bacc(None, target_bir_lowering=False, debug=False, num_devices=R,
                   num_swdge_queues=NQ)

    xT = nc.dram_tensor("xT", [P, TILES * KC * P], b16, kind="ExternalInput")
    wdT = nc.dram_tensor("wdT", [P, KC * EMB], b16, kind="ExternalInput")
    bd = nc.dram_tensor("bd", [EMB, 1], fp32, kind="ExternalInput")
    wgT = nc.dram_tensor("wgT", [EMB, OUT], b16, kind="ExternalInput")
    att = nc.dram_tensor("att", [OUT, 2], b16, kind="ExternalInput")
    bgat = nc.dram_tensor("bgat", [OUT, 1], fp32, kind="ExternalInput")
    offs_ext = [
        nc.dram_tensor("offsA", [P, 8 * TOT[0]], i16, kind="ExternalInput"),
        nc.dram_tensor("offsB", [P, 8 * TOT[1]], i16, kind="ExternalInput"),
    ]
    mask_ext = [
        nc.dram_tensor("maskA", [P, TOT[0]], b16, kind="ExternalInput"),
        nc.dram_tensor("maskB", [P, TOT[1]], b16, kind="ExternalInput"),
    ]
    out = nc.dram_tensor("out", [NL_pad, OUT], fp32, kind="ExternalOutput")

    with tile.TileContext(nc) as tc:
        with (
            tc.tile_pool(name="dram", bufs=1, space="DRAM") as dram,
            tc.tile_pool(name="const", bufs=1) as cst,
        ):
            shards = [dram.tile([H, W_ROW], b16, name=f"shard{i}")
                      for i in range(2)]
            tables = [dram.tile([NHALF, W_ROW], b16, addr_space="Shared",
                                name=f"table{i}")
                      for i in range(2)]

            identb = cst.tile([P, P], b16)
            make_identity(nc, identb[:])
            identf = cst.tile([P, P], fp32)
            make_identity(nc, identf[:])

            wdTs = cst.tile([P, KC * EMB], b16)
            nc.sync.dma_start(out=wdTs[:], in_=wdT[:, :])
            bds = cst.tile([EMB, 1], fp32)
            nc.sync.dma_start(out=bds[:], in_=bd[:, :])
            wgTs = cst.tile([EMB, OUT], b16)
            nc.sync.dma_start(out=wgTs[:], in_=wgT[:, :])
            atts = cst.tile([OUT, 2], b16)
            nc.sync.dma_start(out=atts[:], in_=att[:, :])
            bgs = cst.tile([OUT, 1], fp32)
            nc.sync.dma_start(out=bgs[:], in_=bgat[:, :])
            adst_all = cst.tile([P, TILES], fp32)
            adst2_all = cst.tile([P, TILES], fp32)

            # preload gather indices + pad masks (pure inputs)
            oidx = [cst.tile([P, 8 * TOT[h]], i16, name=f"oidx{h}")
                    for h in range(2)]
            masks = [cst.tile([P, TOT[h]], b16, name=f"mask{h}")
                     for h in range(2)]
            for h in range(2):
                nc.sync.dma_start(out=oidx[h][:], in_=offs_ext[h][:, :])
                nc.sync.dma_start(out=masks[h][:], in_=mask_ext[h][:, :])

            shard_f32 = [s[:].bitcast(fp32) for s in shards]  # [H, 64]

            # ---------------- node phase ----------------
            gloc = cst.tile([P, TILES * OUT], b16)
            asrc_all = cst.tile([P, TILES], fp32)
            with (
                tc.tile_pool(name="npsum_h", bufs=2, space="PSUM") as ps_h,
                tc.tile_pool(name="npsum_m", bufs=1, space="PSUM") as ps_m,
                tc.tile_pool(name="nsb", bufs=4) as nsb,
            ):
                bgp = ps_m.tile([P, OUT], fp32, tag="misc")
                nc.tensor.transpose(out=bgp[:], in_=bgs[:].to_broadcast([OUT, P]),
                                    identity=identf[:OUT, :OUT])
                bgmat = cst.tile([P, OUT], fp32)
                nc.vector.tensor_copy(bgmat[:], bgp[:])

                for t in range(TILES):
                    hh = 0 if t < HTILES else 1
                    row0 = t * P - hh * H
                    xTs = nsb.tile([P, KC * P], b16, tag="xTs")
                    nc.sync.dma_start(
                        out=xTs[:], in_=xT[:, t * KC * P:(t + 1) * KC * P])
                    hTp = ps_h.tile([EMB, P], fp32, tag="hT")
                    for k in range(KC):
                        nc.tensor.matmul(out=hTp[:],
                                         lhsT=wdTs[:, k * EMB:(k + 1) * EMB],
                                         rhs=xTs[:, k * P:(k + 1) * P],
                                         start=(k == 0), stop=(k == KC - 1))
                    hT = nsb.tile([EMB, P], b16, tag="hT_sb")
                    nc.scalar.activation(hT[:], hTp[:],
                                         mybir.ActivationFunctionType.Lrelu,
                                         bias=bds[:, :1])
                    gTp = ps_m.tile([OUT, P], fp32, tag="misc")
                    nc.tensor.matmul(out=gTp[:], lhsT=wgTs[:], rhs=hT[:],
                                     start=True, stop=True)
                    stg = nsb.tile([OUT, P], b16, tag="stg")
                    nc.vector.tensor_copy(stg[:], gTp[:])
                    app = ps_m.tile([2, P], fp32, tag="app")
                    nc.tensor.matmul(out=app[:], lhsT=atts[:], rhs=stg[:],
                                     start=True, stop=True)
                    # transpose gT -> table g block
                    ttp = ps_m.tile([P, OUT], b16, tag="ttp")
                    nc.tensor.transpose(out=ttp[:], in_=stg[:],
                                        identity=identb[:OUT, :OUT])
                    row = nsb.tile([P, 2 * A_SRC_F32 + 4], b16, tag="row")
                    nc.scalar.activation(row[:, 0:OUT], ttp[:],
                                         mybir.ActivationFunctionType.Copy)
                    nc.vector.tensor_copy(gloc[:, t * OUT:(t + 1) * OUT],
                                          row[:, 0:OUT])
                    # transpose [a_src; a_dst] -> [P, 2] f32
                    aps = nsb.tile([2, P], fp32, tag="aps")
                    nc.vector.tensor_copy(aps[:], app[:])
                    atp = ps_m.tile([P, 2], fp32, tag="atp")
                    nc.tensor.transpose(out=atp[:], in_=aps[:],
                                        identity=identf[:2, :2])
                    row_f32 = row[:].bitcast(fp32)
                    nc.vector.tensor_copy(row_f32[:, A_SRC_F32:A_SRC_F32 + 2],
                                          atp[:])
                    nc.vector.tensor_copy(adst_all[:, t:t + 1],
                                          row_f32[:, A_SRC_F32 + 1:
                                                  A_SRC_F32 + 2])
                    nc.vector.tensor_copy(asrc_all[:, t:t + 1],
                                          row_f32[:, A_SRC_F32:A_SRC_F32 + 1])
                    nc.scalar.dma_start(
                        out=shards[hh][row0:row0 + P, 0:2 * A_SRC_F32 + 4],
                        in_=row[:])
                    if t == HTILES - 1:
                        nc.gpsimd.collective_compute(
                            "AllGather", mybir.AluOpType.bypass,
                            replica_groups=[list(range(R))],
                            ins=[shards[0].opt()], outs=[tables[0].opt()],
                        )

                nc.scalar.activation(adst2_all[:], adst_all[:],
                                     mybir.ActivationFunctionType.Copy,
                                     scale=0.2)

            # ---------------- edge phase: two sweeps ----------------
            qi = 0
            with (
                tc.tile_pool(name="acc", bufs=1) as accp,
                tc.tile_pool(name="egp", bufs=6) as egp,
                tc.tile_pool(name="esb", bufs=3) as esb,
            ):
                numA = [accp.tile([P, OUT], fp32, name=f"numA{t}")
                        for t in range(TILES)]
                denA = [accp.tile([P, 1], fp32, name=f"denA{t}")
                        for t in range(TILES)]

                first_sweep = True
                for h in range(2):
                    cum = 0
                    for t in range(TILES):
                        if first_sweep:
                            nc.gpsimd.collective_compute(
                                "AllGather", mybir.AluOpType.bypass,
                                replica_groups=[list(range(R))],
                                ins=[shards[1].opt()],
                                outs=[tables[1].opt()],
                            )
                            first_sweep = False
                        D = int(D_lists[h][t])
                        adst = adst_all[:, t:t + 1]
                        adst02 = adst2_all[:, t:t + 1]
                        gpad = egp.tile([P, D * W_ROW], b16, tag="gp")
                        for j in range(0, D, DCALL):
                            Dj = min(DCALL, D - j)
                            nc.gpsimd.dma_gather(
                                out_ap=gpad[:, j * W_ROW:(j + Dj) * W_ROW]
                                    .rearrange("p (d w) -> p d w", w=W_ROW),
                                in_ap=tables[h][:, :],
                                idxs_ap=oidx[h][:, 8 * (cum + j):
                                                8 * (cum + j + Dj)],
                                num_idxs=P * Dj, num_idxs_reg=P * Dj,
                                elem_size=W_ROW,
                                queue_num=qi % NQ,
                            )
                            qi += 1
                        asrc = gpad[:].bitcast(fp32).rearrange(
                            "p (d w) -> p d w", w=W_ROW // 2)[:, :, A_SRC_F32]
                        t1 = esb.tile([P, D], fp32, tag="t1")
                        nc.scalar.activation(t1[:], asrc,
                                             mybir.ActivationFunctionType.Exp,
                                             bias=adst, scale=1.0)
                        t2 = esb.tile([P, D], fp32, tag="t2")
                        nc.scalar.activation(t2[:], asrc,
                                             mybir.ActivationFunctionType.Exp,
                                             bias=adst02, scale=0.2)
                        Traw = esb.tile([P, D], fp32, tag="Traw")
                        nc.vector.tensor_tensor(out=Traw[:], in0=t1[:],
                                                in1=t2[:],
                                                op=mybir.AluOpType.max)
                        # mask pads, then reduce for the denominator partial
                        Tm = esb.tile([P, D], b16, tag="Tm")
                        nc.vector.tensor_tensor(out=Tm[:], in0=Traw[:],
                                                in1=masks[h][:, cum:cum + D],
                                                op=mybir.AluOpType.mult)
                        den = denA[t] if h == 0 else esb.tile([P, 1], fp32,
                                                              tag="den")
                        nc.vector.tensor_reduce(out=den[:], in_=Tm[:],
                                                op=mybir.AluOpType.add,
                                                axis=mybir.AxisListType.X)
                        # weighted rows
                        gsc = esb.tile([P, D * OUT], b16, tag="gsc")
                        nc.vector.tensor_tensor(
                            out=gsc[:].rearrange("p (d c) -> p d c", c=OUT),
                            in0=gpad[:].rearrange("p (d w) -> p d w",
                                                  w=W_ROW)[:, :, 0:OUT],
                            in1=Tm[:].to_broadcast([P, D, OUT]),
                            op=mybir.AluOpType.mult)
                        num = numA[t] if h == 0 else esb.tile([P, OUT], fp32,
                                                              tag="num")
                        nc.vector.tensor_reduce(
                            out=num[:],
                            in_=gsc[:].rearrange("p (d c) -> p c d", c=OUT),
                            op=mybir.AluOpType.add, axis=mybir.AxisListType.X)
                        if h == 1:
                            # self-loop contribution from the local g tile
                            s1 = esb.tile([P, 1], fp32, tag="s1")
                            nc.scalar.activation(
                                s1[:], asrc_all[:, t:t + 1],
                                mybir.ActivationFunctionType.Exp,
                                bias=adst, scale=1.0)
                            s2 = esb.tile([P, 1], fp32, tag="s2")
                            nc.scalar.activation(
                                s2[:], asrc_all[:, t:t + 1],
                                mybir.ActivationFunctionType.Exp,
                                bias=adst02, scale=0.2)
                            wself = esb.tile([P, 1], fp32, tag="wself")
                            nc.vector.tensor_tensor(out=wself[:], in0=s1[:],
                                                    in1=s2[:],
                                                    op=mybir.AluOpType.max)
                            stot = esb.tile([P, OUT], fp32, tag="stot")
                            nc.vector.tensor_tensor(out=stot[:], in0=num[:],
                                                    in1=numA[t][:],
                                                    op=mybir.AluOpType.add)
                            stot2 = esb.tile([P, OUT], fp32, tag="stot2")
                            nc.vector.scalar_tensor_tensor(
                                out=stot2[:],
                                in0=gloc[:, t * OUT:(t + 1) * OUT],
                                scalar=wself[:, :1], in1=stot[:],
                                op0=mybir.AluOpType.mult,
                                op1=mybir.AluOpType.add)
                            dtot = esb.tile([P, 1], fp32, tag="dtot")
                            nc.vector.tensor_tensor(out=dtot[:], in0=den[:],
                                                    in1=denA[t][:],
                                                    op=mybir.AluOpType.add)
                            dtot2 = esb.tile([P, 1], fp32, tag="dtot2")
                            nc.vector.tensor_tensor(out=dtot2[:], in0=dtot[:],
                                                    in1=wself[:],
                                                    op=mybir.AluOpType.add)
                            rden = esb.tile([P, 1], fp32, tag="rden")
                            nc.vector.reciprocal(rden[:], dtot2[:])
                            outf = esb.tile([P, OUT], fp32, tag="outf")
                            nc.vector.scalar_tensor_tensor(
                                out=outf[:], in0=stot2[:], scalar=rden[:, :1],
                                in1=bgmat[:],
                                op0=mybir.AluOpType.mult,
                                op1=mybir.AluOpType.add)
                            nc.sync.dma_start(out=out[t * P:(t + 1) * P, :],
                                              in_=outf[:])
                        cum += D
    nc.finalize()
    return nc


# --------------------------------------------------------------------------
# entry points
# --------------------------------------------------------------------------

def run(inputs, cfg=CFG_REAL, trace=False):
    from concourse.bass_utils import run_bass_kernel_spmd
    in_maps, orders, D_lists = _prepare(cfg, **inputs)
    nc = _build_graph(cfg, D_lists)
    res = run_bass_kernel_spmd(nc, in_maps, core_ids=list(range(R)),
                               trace=trace)
    out = _assemble(cfg, res.results, orders)
    return out, res


def kernel(**inputs):
    inputs = {k: np.asarray(v) for k, v in inputs.items()}
    out, _ = run(inputs, CFG_REAL, trace=False)
    return out
